# revision 29
# baseline (speedup 1.0000x reference)
"""Multi-head attention (B=4, S=2048, D=512, H=8) on 8 Trainium2 cores.

Sharding: core c = (batch b = c//2, query-half = c%2). Each core computes
1024 query rows of one batch over all 2048 keys and all 8 heads, producing
a disjoint slice of the output -> no inter-core reduction needed.

Per-core layout is fully "transposed land" (contraction dim on partitions):
  xT [512,1024], yT [512,2048] prepared (transposed, bf16) on host.
  QT = Wq^T @ xT   (Wq pre-scaled by depth^-0.5 on host)
  KT = Wk^T @ yT
  V  = y @ Wv in natural [keys, dim] layout, stored strided into
       V_aug [128, 8*65] with a ones column per head (row 64 of the
       attention matmul output then accumulates softmax denominators).

Schedule (v2 — ScalarE exp is the bottleneck engine at ~142us busy, so
everything is organized to start it early and never starve it):
  - DMA priority: wk+yT first, then wq+xT, then wv, wo.
  - Only KT/QT for head pair 0 are computed up front; first exp issues
    ~15us in. V tiles and later pairs' KT/QT projections are emitted
    inside the attention loops where the PE has slack (ScalarE-bound
    steady state leaves ~40% PE idle per iteration).
  - per head pair (2p, 2p+1): head A on partitions 0:64, head B on
    64:128 of shared KT/QT tiles; their logits matmuls target disjoint
    PE row groups and run concurrently.
       logits[kt,qb] = (KT tile)^T @ QT  (bf16 operands, fp32 PSUM)
       exp over [128, 1024] (ScalarE, PSUM -> SBUF bf16)
       attnT += V_aug^T @ PT, fp32 PSUM, accumulated over 16 key tiles.
  - pair-end normalization: evacuate both heads' [65,1024] PSUM to SBUF
    (releases the psum banks for the next pair), then per head:
    reciprocal_approx_fast on the denominator row (single DVE op, ~51
    ULP — vs ~7us for the exact iterative reciprocal), gpsimd
    partition_broadcast, DVE multiply -> attnT bf16. All off the
    critical path except for the last pair.
  - out = attnT^T @ Wo per 128-query tile -> DMA (fp32).
Softmax skips max-subtraction (logits ~ N(0,1); exp cannot overflow fp32).
Matmul operands are bf16 (1 cycle/row on the PE vs 2 for fp32); all PSUM
accumulation fp32. End-to-end RMS relative error vs fp32 ~4e-3.
"""

import numpy as np
import ml_dtypes

import concourse.bass as bass
import concourse.tile as tile
from concourse import bacc, mybir
from concourse.bass_utils import run_bass_kernel_spmd

F32 = mybir.dt.float32
BF16 = mybir.dt.bfloat16
EXP = mybir.ActivationFunctionType.Exp

B, S, D = 4, 2048, 512
H = 8
DEPTH = D // H  # 64
SQ = S // 2  # queries per core (1024)
SK = S  # keys per core (2048)
N_CORES = 8

P = 128
KT4 = D // P  # 4 contraction tiles for projections
NKT = SK // P  # 16 key tiles
NQT = SQ // P  # 8 query tiles
VAUG_W = H * (DEPTH + 1)  # 520


def build_nc():
    nc = bacc.Bacc("TRN2", target_bir_lowering=False, debug=False)

    xT = nc.dram_tensor("xT", [D, SQ], BF16, kind="ExternalInput").ap()
    yT = nc.dram_tensor("yT", [D, SK], BF16, kind="ExternalInput").ap()
    wq = nc.dram_tensor("wq", [D, D], BF16, kind="ExternalInput").ap()
    wk = nc.dram_tensor("wk", [D, D], BF16, kind="ExternalInput").ap()
    wv = nc.dram_tensor("wv", [D, D], BF16, kind="ExternalInput").ap()
    wo = nc.dram_tensor("wo", [D, D], BF16, kind="ExternalInput").ap()
    out = nc.dram_tensor("out", [SQ, D], F32, kind="ExternalOutput").ap()

    with tile.TileContext(nc) as tc:
        with (
            tc.tile_pool(name="acts", bufs=1) as apool,
            tc.tile_pool(name="ps", bufs=1, space="PSUM") as pspool,
            tc.tile_pool(name="pt", bufs=8) as ptpool,
            tc.tile_pool(name="small", bufs=2) as spool,
            tc.tile_pool(name="outsb", bufs=2) as opool,
        ):
            # ---- HAM warmup: the PE clock-gate opens only after ~3.4us of
            # sustained matmul activity, and the whole prologue otherwise
            # runs at the cold 1.2 GHz. Spam cheap N=64 matmuls into a
            # scratch PSUM slot while the first DMAs are in flight.
            warm = apool.tile([P, 64], BF16, name="warm", tag="warm", bufs=1)
            nc.vector.memset(warm[:], 0.0)
            wps = pspool.tile([P, SQ], F32, name="wps", tag="lg", bufs=3)
            for i in range(60):
                nc.tensor.matmul(wps[0:64, 0:64], warm[:], warm[:],
                                 start=True, stop=True)
            warm_anchor = apool.tile([1, 64], F32, name="warma", tag="warma", bufs=1)
            nc.vector.tensor_copy(warm_anchor[:], wps[0:1, 0:64])

            # ---- load inputs, in the order the compute needs them ----
            # Alternate between the two HWDGE queues (SP + ScalarE) so the
            # prologue-critical tensors stream in parallel; one queue alone
            # caps input DMA at ~330 GB/s with ~8.5us of startup latency.
            def load4(name, src, width, tiles=None, col0=0):
                made = tiles is None
                if made:
                    tiles = []
                for k in range(KT4):
                    if made:
                        t = apool.tile(
                            [P, width], BF16, name=f"{name}{k}", tag=f"{name}{k}"
                        )
                        tiles.append(t)
                    eng = nc.sync if k % 2 == 0 else nc.scalar
                    eng.dma_start(
                        tiles[k][:, col0 : col0 + width],
                        src[k * P : (k + 1) * P, col0 : col0 + width],
                    )
                return tiles

            wk_sb = load4("wk", wk, D)
            yT_sb = [
                apool.tile([P, SK], BF16, name=f"yt{k}", tag=f"yt{k}")
                for k in range(KT4)
            ]
            load4("yt", yT, SQ, tiles=yT_sb, col0=0)  # first key half
            wv_sb = load4("wv", wv, D)
            wq_sb = load4("wq", wq, D)
            xT_sb = load4("xt", xT, SQ)
            load4("yt", yT, SQ, tiles=yT_sb, col0=SQ)  # second key half
            wo_sb = load4("wo", wo, D)

            ones_sb = apool.tile([P, H], F32, name="ones_sb", tag="ones", bufs=1)
            nc.vector.memset(ones_sb[:], 1.0)
            ones_v = ones_sb.rearrange("p (h c) -> p h c", h=H, c=1)

            # ---- projection emitters (each borrows one 'lg' psum slot) ----
            V_sb = [None] * NKT

            def emit_v(kt):
                t = apool.tile([P, VAUG_W], BF16, name=f"vaug{kt}", tag=f"vaug{kt}")
                ps = pspool.tile([P, SQ], F32, name=f"vps{kt}", tag="lg", bufs=3)
                for k in range(KT4):
                    nc.tensor.matmul(
                        ps[:, :512],
                        yT_sb[k][:, kt * P : (kt + 1) * P],
                        wv_sb[k][:],
                        start=(k == 0),
                        stop=(k == KT4 - 1),
                    )
                tv = t.rearrange("p (h c) -> p h c", h=H, c=DEPTH + 1)
                nc.vector.tensor_copy(
                    tv[:, :, 0:DEPTH],
                    ps[:, :512].rearrange("p (h c) -> p h c", h=H, c=DEPTH),
                )
                nc.vector.tensor_copy(tv[:, :, DEPTH : DEPTH + 1], ones_v)
                V_sb[kt] = t

            QT_sb = [None] * KT4
            KT_sb = [None] * KT4

            def emit_kt_part(p, kb, qb, state):
                # One 4-matmul quarter of the K projection + its own 512-col
                # copy. Kept small so a hook never drains the 2-deep exp
                # buffer (8-matmul blocks at the cold clock cost ~3.4us), and
                # so the DVE copies land well before the pair boundary.
                if KT_sb[p] is None:
                    KT_sb[p] = apool.tile(
                        [P, SK], BF16, name=f"ktsb{p}", tag=f"ktsb{p}"
                    )
                t = KT_sb[p]
                if qb == 0:
                    state["ps"] = pspool.tile(
                        [P, SQ], F32, name=f"ktps{p}_{kb}", tag="lg", bufs=3
                    )
                ps = state["ps"]
                for k in range(KT4):
                    nc.tensor.matmul(
                        ps[:, qb * 512 : (qb + 1) * 512],
                        wk_sb[k][:, p * P : (p + 1) * P],
                        yT_sb[k][
                            :,
                            kb * SQ + qb * 512 : kb * SQ + (qb + 1) * 512,
                        ],
                        start=(k == 0),
                        stop=(k == KT4 - 1),
                    )
                nc.vector.tensor_copy(
                    t[:, kb * SQ + qb * 512 : kb * SQ + (qb + 1) * 512],
                    ps[:, qb * 512 : (qb + 1) * 512],
                )

            def emit_qt_part(p, qb, state):
                if QT_sb[p] is None:
                    QT_sb[p] = apool.tile(
                        [P, SQ], BF16, name=f"qtsb{p}", tag=f"qtsb{p}"
                    )
                if qb == 0:
                    state["ps"] = pspool.tile(
                        [P, SQ], F32, name=f"qtps{p}", tag="lg", bufs=3
                    )
                ps = state["ps"]
                for k in range(KT4):
                    nc.tensor.matmul(
                        ps[:, qb * 512 : (qb + 1) * 512],
                        wq_sb[k][:, p * P : (p + 1) * P],
                        xT_sb[k][:, qb * 512 : (qb + 1) * 512],
                        start=(k == 0),
                        stop=(k == KT4 - 1),
                    )
                nc.vector.tensor_copy(
                    QT_sb[p][:, qb * 512 : (qb + 1) * 512],
                    ps[:, qb * 512 : (qb + 1) * 512],
                )

            def emit_kt_half(p, kb):
                state = {}
                emit_kt_part(p, kb, 0, state)
                emit_kt_part(p, kb, 1, state)

            def emit_qt(p):
                state = {}
                emit_qt_part(p, 0, state)
                emit_qt_part(p, 1, state)

            def kqt_parts(p):
                """Six sub-hooks that together build KT[p] and QT[p]."""
                s1, s2, s3 = {}, {}, {}
                return [
                    lambda: emit_kt_part(p, 0, 0, s1),
                    lambda: emit_kt_part(p, 0, 1, s1),
                    lambda: emit_kt_part(p, 1, 0, s2),
                    lambda: emit_kt_part(p, 1, 1, s2),
                    lambda: emit_qt_part(p, 0, s3),
                    lambda: emit_qt_part(p, 1, s3),
                ]

            # ---- prologue: only pair 0's KT/QT, plus the first V tiles.
            # The first two logits/exp pairs are emitted BEFORE the V
            # projections: the PE queue is strict FIFO, so anything emitted
            # earlier delays the first exp.
            emit_kt_half(0, 0)
            emit_qt(0)

            attnT_sb = []
            for p in range(KT4):
                t = apool.tile([P, SQ], BF16, name=f"attnt{p}", tag=f"attnt{p}")
                attnT_sb.append(t)

            # ---- output projection emitter (per 128-query tile) ----
            # qt 0..3 read only phase-0 columns of attnT and are emitted
            # inside pair 3 phase 1, overlapping the last attention phase.
            def emit_oproj(qt):
                ps = pspool.tile([P, 512], F32, name=f"ops{qt}", tag="lg", bufs=3)
                for k in range(KT4):
                    nc.tensor.matmul(
                        ps[:, :512],
                        attnT_sb[k][:, qt * P : (qt + 1) * P],
                        wo_sb[k][:],
                        start=(k == 0),
                        stop=(k == KT4 - 1),
                    )
                osb = opool.tile([P, D], F32, name=f"osb{qt}", tag="osb")
                nc.vector.tensor_copy(osb[:], ps[:, :512])
                nc.sync.dma_start(out[qt * P : (qt + 1) * P, :], osb[:])

            # Projection / output work hosted inside the attention loops,
            # keyed by (pair, phase, kt). Each borrows the third 'lg' slot.
            hooks = {}

            def add_hook(key, fn):
                hooks.setdefault(key, []).append(fn)

            # V tiles 1..15 spread through pair 0 phase 0 (V[j] is consumed
            # by PV at iteration j, so emit it 1-2 iterations earlier).
            vsched = {0: (4,), 1: (5,), 2: (6,), 5: (7, 8), 6: (9,),
                      7: (10, 11), 8: (12,), 9: (13, 14), 10: (15,)}
            for kt, vs in vsched.items():
                for j in vs:
                    add_hook((0, 0, kt), (lambda v=j: emit_v(v)))
            # keys 1024:2048 of pair 0 — deferred so its matmuls don't sit in
            # the PE FIFO waiting on the second-half yT DMA.
            s01 = {}
            add_hook((0, 0, 3), lambda: emit_kt_part(0, 1, 0, s01))
            add_hook((0, 0, 4), lambda: emit_kt_part(0, 1, 1, s01))
            # next pair's KT/QT, six 4-matmul sub-hooks each
            for kt, fn in zip((1, 3, 5, 7, 9, 11), kqt_parts(1)):
                add_hook((0, 1, kt), fn)
            for pr in (1, 2):
                for kt, fn in zip((3, 5, 7, 9, 11, 13), kqt_parts(pr + 1)):
                    add_hook((pr, 0, kt), fn)
            for i, kt in enumerate((6, 8, 10, 12)):
                add_hook((3, 1, kt), (lambda q=i: emit_oproj(q)))

            # ---- attention: head-pair outer, query-phase (512 q) middle ----
            # With the query dim split into two 512-wide phases, the two
            # attention accumulators are [65,512] = one PSUM bank each, which
            # frees enough PSUM for THREE logits slots. The third slot is what
            # lets the V / KT / QT projection borrows proceed without ever
            # blocking the logits->exp stream (strict-FIFO engine queues turn
            # any slot wait into a ScalarE bubble).
            def emit_logits(pr, phase, kt):
                q0 = phase * 512
                lg = pspool.tile(
                    [P, SQ], F32, name=f"lg{pr}_{phase}_{kt}", tag="lg", bufs=3
                )
                for half in range(2):
                    nc.tensor.matmul(
                        lg[:, half * 512 : (half + 1) * 512],
                        KT_sb[pr][
                            half * DEPTH : (half + 1) * DEPTH,
                            kt * P : (kt + 1) * P,
                        ],
                        QT_sb[pr][
                            half * DEPTH : (half + 1) * DEPTH, q0 : q0 + 512
                        ],
                        start=True,
                        stop=True,
                    )
                pt = ptpool.tile(
                    [P, SQ], BF16, name=f"pt{pr}_{phase}_{kt}", tag="pt"
                )
                nc.scalar.activation(pt[:], lg[:], EXP)
                return pt

            def emit_norm(pr, phase, attn_ph):
                # ---- phase-end normalization ----
                # Evacuate both heads' PSUM first (releases the 'at' banks
                # for the next phase), then a single batched DVE reciprocal
                # (both heads' denominator rows gathered to partitions 0/32
                # of one tile; the iterative reciprocal is free-dim-bound so
                # one [33,512] op costs the same as [1,512]) runs off the
                # critical path. The very last phase instead multiplies
                # straight out of PSUM (no evac needed at kernel end) with a
                # ScalarE exp(-ln(x)) reciprocal (same table set family as
                # the softmax exp, ~9e-6 rel err): ScalarE is idle once the
                # final exp retires and its chain is shorter.
                q0 = phase * 512
                last = pr == KT4 - 1 and phase == 1
                auns = []
                for half in range(2):
                    h = 2 * pr + half
                    aun = spool.tile(
                        [DEPTH + 1, 512], F32, name=f"aun{h}_{phase}",
                        tag=f"aun{half}",
                    )
                    nc.vector.tensor_copy(aun[:], attn_ph[half][:, :])
                    auns.append(aun)
                recips = []
                if last:
                    for half in range(2):
                        lnd = spool.tile(
                            [1, 512], F32, name=f"ln_den{half}", tag="lnden",
                            bufs=2,
                        )
                        nc.scalar.activation(
                            lnd[:], attn_ph[half][DEPTH : DEPTH + 1, :],
                            mybir.ActivationFunctionType.Ln,
                        )
                        recip = spool.tile(
                            [1, 512], F32, name=f"recipl{half}",
                            tag=f"recip{half}",
                        )
                        nc.scalar.activation(recip[:], lnd[:], EXP, scale=-1.0)
                        recips.append(recip[:])
                else:
                    dens = spool.tile(
                        [33, 512], F32, name=f"dens{pr}_{phase}", tag="dens",
                        bufs=2,
                    )
                    nc.gpsimd.memset(dens[:], 1.0)
                    nc.vector.tensor_copy(
                        dens[0:1, :], auns[0][DEPTH : DEPTH + 1, :]
                    )
                    nc.vector.tensor_copy(
                        dens[32:33, :], auns[1][DEPTH : DEPTH + 1, :]
                    )
                    drec = spool.tile(
                        [33, 512], F32, name=f"drec{pr}_{phase}", tag="drec",
                        bufs=2,
                    )
                    nc.vector.reciprocal(drec[:], dens[:])
                    recipb = spool.tile(
                        [1, 512], F32, name=f"recipb{pr}_{phase}", tag="recip1"
                    )
                    nc.vector.tensor_copy(recipb[:], drec[32:33, :])
                    recips = [drec[0:1, :], recipb[:]]

                def back(pr=pr, phase=phase, auns=auns, recips=recips):
                    # Deferred ~4 iterations into the following phase: the
                    # DVE multiply waits on the gpsimd broadcast, and if it
                    # sat in the strict-FIFO DVE queue right at the boundary
                    # it would block the next phase's projection copies
                    # behind a cross-engine wait.
                    for half in range(2):
                        h = 2 * pr + half
                        dst = attnT_sb[pr][
                            half * DEPTH : (half + 1) * DEPTH,
                            phase * 512 : phase * 512 + 512,
                        ]
                        bcast = spool.tile(
                            [DEPTH, 512], F32, name=f"bcast{h}_{phase}",
                            tag=f"bcast{half}",
                        )
                        nc.gpsimd.partition_broadcast(bcast[:], recips[half])
                        nc.vector.tensor_mul(
                            dst, auns[half][0:DEPTH, :], bcast[:]
                        )

                return back

            # ---- attention: one flat pipeline over (pair, phase, kt) ----
            # Logits+exp always run 2 iterations ahead of PV — across phase
            # and pair boundaries too — so the exp stream never drains: a
            # drained pipeline costs ~2-3.5us per boundary (PVs trail the
            # last exp and the next phase's logits sit behind them in the
            # strict PE FIFO, re-throttling the HAM clock gate on top).
            iters = [
                (pr, phase, kt)
                for pr in range(KT4)
                for phase in range(2)
                for kt in range(NKT)
            ]
            pts = {}
            pts[iters[0]] = emit_logits(*iters[0])
            pts[iters[1]] = emit_logits(*iters[1])
            # first V tiles ride behind the first logits in the PE FIFO,
            # filling the exp(0)/exp(1) wait window
            for j in range(4):
                emit_v(j)
            attn_cur = {}
            for i, (pr, phase, kt) in enumerate(iters):
                if kt == 0:
                    attn_cur[(pr, phase)] = [
                        pspool.tile(
                            [DEPTH + 1, 512], F32,
                            name=f"attnps{2 * pr + half}_{phase}",
                            tag="at", bufs=2,
                        )
                        for half in range(2)
                    ]
                attn_ph = attn_cur[(pr, phase)]
                pt = pts.pop((pr, phase, kt))
                # Emit the lookahead logits BEFORE this iteration's PV: at a
                # phase boundary PV(kt=0) waits on the previous phase's PSUM
                # evacuation, and anything queued behind it on the PE would
                # stall the exp stream.
                if i + 2 < len(iters):
                    pts[iters[i + 2]] = emit_logits(*iters[i + 2])
                for half in range(2):
                    h = 2 * pr + half
                    nc.tensor.matmul(
                        attn_ph[half][:, :],
                        V_sb[kt][:, h * (DEPTH + 1) : (h + 1) * (DEPTH + 1)],
                        pt[:, half * 512 : (half + 1) * 512],
                        start=(kt == 0),
                        stop=(kt == NKT - 1),
                    )
                for hook in hooks.get((pr, phase, kt), ()):
                    hook()
                if kt == NKT - 1:
                    back = emit_norm(pr, phase, attn_ph)
                    if i + 5 < len(iters):
                        key = iters[i + 5]
                        hooks.setdefault(key, []).append(back)
                    else:
                        back()

            # ---- output projection tail: phase-1 query tiles. The pair 0-2
            # contributions are emitted first on their own PSUM borrows so
            # they run during the last phase's normalization chain; only the
            # pair-3 matmul + evacuation waits for the final attnT chunk.
            tail_ps = []
            for qt in range(NQT // 2, NQT):
                tag = "lg" if qt < NQT - 1 else "at"
                ps = pspool.tile(
                    [P, 512], F32, name=f"ops{qt}", tag=tag,
                    bufs=(3 if tag == "lg" else 2),
                )
                for k in range(KT4 - 1):
                    nc.tensor.matmul(
                        ps[:, :512],
                        attnT_sb[k][:, qt * P : (qt + 1) * P],
                        wo_sb[k][:],
                        start=(k == 0),
                        stop=False,
                    )
                tail_ps.append(ps)
            for qt in range(NQT // 2, NQT):
                ps = tail_ps[qt - NQT // 2]
                nc.tensor.matmul(
                    ps[:, :512],
                    attnT_sb[KT4 - 1][:, qt * P : (qt + 1) * P],
                    wo_sb[KT4 - 1][:],
                    start=False,
                    stop=True,
                )
                osb = opool.tile([P, D], F32, name=f"osb{qt}", tag="osb")
                nc.vector.tensor_copy(osb[:], ps[:, :512])
                nc.sync.dma_start(out[qt * P : (qt + 1) * P, :], osb[:])

    nc.compile()
    return nc


_CACHE: dict = {}


def get_nc():
    if "nc" not in _CACHE:
        _CACHE["nc"] = build_nc()
    return _CACHE["nc"]


def make_in_maps(x, y, W_q, W_k, W_v, W_o):
    bf = ml_dtypes.bfloat16
    x = np.ascontiguousarray(x, dtype=np.float32)
    y = np.ascontiguousarray(y, dtype=np.float32)
    wq = (np.asarray(W_q, dtype=np.float32) * np.float32(DEPTH**-0.5)).astype(bf)
    wk = np.asarray(W_k, dtype=np.float32).astype(bf)
    wv = np.asarray(W_v, dtype=np.float32).astype(bf)
    wo = np.asarray(W_o, dtype=np.float32).astype(bf)
    yT_cache = [np.ascontiguousarray(y[b].T).astype(bf) for b in range(B)]
    in_maps = []
    for c in range(N_CORES):
        b, half = c // 2, c % 2
        in_maps.append(
            {
                "xT": np.ascontiguousarray(
                    x[b, half * SQ : (half + 1) * SQ, :].T
                ).astype(bf),
                "yT": yT_cache[b],
                "wq": wq,
                "wk": wk,
                "wv": wv,
                "wo": wo,
            }
        )
    return in_maps


def assemble_out(results):
    out = np.empty((B, S, D), np.float32)
    for c in range(N_CORES):
        b, half = c // 2, c % 2
        out[b, half * SQ : (half + 1) * SQ, :] = results[c]["out"]
    return out


def kernel(x, y, W_q, W_k, W_v, W_o):
    nc = get_nc()
    in_maps = make_in_maps(x, y, W_q, W_k, W_v, W_o)
    res = run_bass_kernel_spmd(nc, in_maps, core_ids=list(range(N_CORES)))
    return assemble_out(res.results)
